# revision 12
# baseline (speedup 1.0000x reference)
"""Two-layer GAT on 8 Trainium2 NeuronCores (Bass/Tile SPMD kernel), v3.

Full inputs in, full output out. Structure:
  - host: bin-pack nodes into (core, tile, row) slots, build per-core edge
    metadata (int16 gather indices wrapped for dma_gather, per-tile local
    dst rows), fold attention vectors into augmented weights, cast to bf16.
    Table slots use an AllGather-chunk-major numbering (s2) so the layer-2
    table is exactly the concatenation of the chunked AllGather outputs
    (no DRAM splice).
  - device (SPMD, 8 cores):
    Phase A (replicated): every core computes the FULL table1
      rows [h1 (256) | al_s (4)] bf16 from a replicated xT — no collective.
    L1 edge phase, per local dst-tile: dma_gather source rows (768B),
      one-hot build on DVE, transpose via PE (copies on ACT), attention
      weights via ACT Prelu -> Relu(C-x) -> Exp(C-x) (exact clamped exp),
      in-place value scaling, PSUM segment sums via one-hot matmuls with
      denominators from an extra matmul, ELU mostly on ACT, layer-2 table
      rows [h2 (64) | 1.0 | als2] bf16 written to tbl2_shard.
      AllGather of tbl2 is split into row-chunks interleaved with the
      tile loop; chunk outputs land directly in tbl2 (s2 ordering).
    L2 edge phase: same machinery on tbl2 (256B rows); the one-hot is
      built pre-scaled by the edge weight (fused is_equal+mult), and the
      constant-1 table column yields the denominator inside the same
      segment matmul.
"""

import heapq
import numpy as np
import ml_dtypes

import concourse.bacc as bacc
import concourse.bass as bass
import concourse.mybir as mybir
import concourse.tile as tile
from concourse.bass_utils import run_bass_kernel_spmd

dt = mybir.dt
f32 = dt.float32
bf16 = dt.bfloat16
npbf16 = ml_dtypes.bfloat16
NEG_SLOPE = 0.2
CLAMP = 60.0


class Cfg:
    def __init__(self, n=50000, f_in=128, heads=4, hid=64, out_ch=64,
                 ncores=8, nt=49, loch=12, hich=8, split=32768):
        self.n = n
        self.f_in = f_in
        self.heads = heads
        self.hid = hid
        self.out_ch = out_ch
        self.ncores = ncores
        self.nt = nt                  # dst tiles per core
        self.ntr = nt * 128           # rows per core
        self.slots = ncores * self.ntr
        self.loch = loch              # lo-src gather chunks per tile
        self.hich = hich
        self.cpt = loch + hich
        self.split = split
        self.t1w = 384                # table1 row width (bf16) = 768B
        self.t2w = 128                # table2 row width (bf16) = 256B
        self.v1 = heads * hid         # 256 value cols (h), als at 256:260
        # tbl2 row: [64 vals | 1.0 | als2]
        self.v2 = out_ch              # 64
        # AG2 tile-chunk boundaries (local tiles)
        self.ag2_bounds = (0, 12, 24, 36, nt)
        assert n <= self.slots
        assert self.split <= 32768
        assert self.slots - self.split <= 32768
        assert self.v1 + heads <= self.t1w
        assert self.v2 + 2 <= self.t2w


FULL = Cfg()


# ---------------------------------------------------------------------------
# Host-side preprocessing
# ---------------------------------------------------------------------------

def pack_nodes(cfg, dst_nodes):
    """Assign each node to a (tile, row) slot, balancing edge counts.

    Tiles whose own rows land in the hi gather range (s2 >= split) carry
    their 128 self-loops in the hi budget, so they get a lower edge cap to
    keep hi-chunk usage under hich*128.
    """
    n, ntile = cfg.n, cfg.ncores * cfg.nt
    deg = np.bincount(dst_nodes, minlength=n)
    order = np.argsort(-deg, kind="stable")
    s2 = slot2_map(cfg)
    selfhi = s2[np.arange(ntile) * 128] >= cfg.split
    # +128 self-loops per tile are not in `deg`; budget them per class.
    lo_cap, hi_cap = cfg.loch * 128, cfg.hich * 128
    flo = cfg.split / cfg.slots
    fhi = 1.0 - flo
    margin = 72
    cap = np.where(
        selfhi,
        128 + (hi_cap - 128 - margin) / fhi,
        128 + (lo_cap - 128 - margin) / flo).astype(np.int64)
    heap = [(0, t) for t in range(ntile)]
    heapq.heapify(heap)
    rows_used = np.zeros(ntile, np.int32)
    tile_of = np.empty(n, np.int32)
    row_of = np.empty(n, np.int32)
    spill = []
    for nd in order:
        d = int(deg[nd])
        while True:
            if not heap:
                # every row-available tile is cap-blocked: place on the
                # least-loaded one anyway (idx build verifies budgets).
                heap = spill
                spill = []
                heapq.heapify(heap)
                while True:
                    l, t = heapq.heappop(heap)
                    if rows_used[t] < 128:
                        break
                break
            l, t = heapq.heappop(heap)
            if rows_used[t] >= 128:
                continue
            if l + 128 + d > cap[t] and l > 0:
                spill.append((l, t))
                continue
            break
        for item in spill:
            heapq.heappush(heap, item)
        spill.clear()
        tile_of[nd] = t
        row_of[nd] = rows_used[t]
        rows_used[t] += 1
        heapq.heappush(heap, (l + d, t))
    perm = tile_of.astype(np.int64) * 128 + row_of
    return perm


def wrap16(a):
    """[nt, slots] int16 -> [128, nt*slots/16] wrapped in 16 partitions,
    replicated to 128."""
    ntl, s = a.shape
    w = a.reshape(ntl, s // 16, 16).transpose(0, 2, 1)   # [nt,16,s/16]
    w = w.transpose(1, 0, 2).reshape(16, ntl * (s // 16))
    return np.tile(w, (8, 1)).copy()


def slot2_map(cfg):
    """Original slot (tile*128+row, tile = c*nt + t) -> s2 slot in
    AllGather-chunk-major order: [chunk k][core c][tile t - b_k][row]."""
    b = np.asarray(cfg.ag2_bounds)
    s = np.arange(cfg.slots)
    g_tile = s // 128
    row = s % 128
    c = g_tile // cfg.nt
    t = g_tile % cfg.nt
    k = np.searchsorted(b, t, side="right") - 1
    off_k = b[:-1] * 128 * cfg.ncores      # row offset where chunk k starts
    ck_tiles = b[1:] - b[:-1]              # tiles per core in chunk k
    s2 = (off_k[k] + (c * ck_tiles[k] + (t - b[k])) * 128 + row)
    return s2.astype(np.int64)


def prep_host(cfg, x, edge_index, W1, a_src1, a_dst1, b1, W2, a_src2, a_dst2, b2):
    n = cfg.n
    heads, hid, out_ch = cfg.heads, cfg.hid, cfg.out_ch
    x = np.asarray(x, np.float32)
    ei = np.asarray(edge_index, np.int64)
    loops = np.arange(n, dtype=np.int64)
    src = np.concatenate([ei[0], loops])
    dst = np.concatenate([ei[1], loops])

    perm = pack_nodes(cfg, dst)
    s2_of_slot = slot2_map(cfg)
    perm2 = s2_of_slot[perm]              # node -> s2 table slot

    s_src = perm2[src]                    # table gathers use s2 numbering
    s_dst = perm[dst]                     # dst tiles stay in original order
    tile_g = s_dst // 128
    r_dst = s_dst % 128
    is_lo = s_src < cfg.split

    nt_all = cfg.ncores * cfg.nt
    lo_slots = cfg.loch * 128
    hi_slots = cfg.hich * 128

    idx_lo = np.zeros((nt_all, lo_slots), np.int16)
    idx_hi = np.zeros((nt_all, hi_slots), np.int16)
    dst_loc = np.full((nt_all, cfg.cpt * 128), -1.0, np.float32)

    key = tile_g * 2 + (~is_lo).astype(np.int64)
    order = np.argsort(key, kind="stable")
    ks = key[order]
    bounds = np.searchsorted(ks, np.arange(2 * nt_all + 1))
    for t in range(nt_all):
        elo = order[bounds[2 * t]:bounds[2 * t + 1]]
        ehi = order[bounds[2 * t + 1]:bounds[2 * t + 2]]
        nlo, nhi = len(elo), len(ehi)
        if nlo > lo_slots or nhi > hi_slots:
            raise RuntimeError(f"tile {t} overflow: lo={nlo} hi={nhi}")
        idx_lo[t, :nlo] = s_src[elo].astype(np.int16)
        idx_hi[t, :nhi] = (s_src[ehi] - cfg.split).astype(np.int16)
        dst_loc[t, :nlo] = r_dst[elo]
        dst_loc[t, lo_slots:lo_slots + nhi] = r_dst[ehi]

    # ---- folded weights ----
    W1 = np.asarray(W1, np.float32)
    W2 = np.asarray(W2, np.float32)
    a_src1 = np.asarray(a_src1, np.float32)
    a_dst1 = np.asarray(a_dst1, np.float32)
    a_src2 = np.asarray(a_src2, np.float32)
    a_dst2 = np.asarray(a_dst2, np.float32)
    Asrc = np.zeros((heads * hid, heads), np.float32)
    Adst = np.zeros((heads * hid, heads), np.float32)
    for h in range(heads):
        Asrc[h * hid:(h + 1) * hid, h] = a_src1[h]
        Adst[h * hid:(h + 1) * hid, h] = a_dst1[h]
    # w1aug: [128, 264] = [W1 (256) | W1@Asrc (4) | W1@Adst (4)]
    w1aug = np.concatenate([W1, W1 @ Asrc, W1 @ Adst], axis=1)

    # w2aug: [256, 66] = [W2 | W2@a_src2 | W2@a_dst2]
    w2aug = np.concatenate([W2, (W2 @ a_src2[0])[:, None],
                            (W2 @ a_dst2[0])[:, None]], axis=1)

    assert not np.any(np.asarray(b1)), "nonzero b1 unsupported"
    assert not np.any(np.asarray(b2)), "nonzero b2 unsupported"

    i128 = np.eye(128, dtype=npbf16)
    iotarow = np.tile(np.arange(128, dtype=npbf16), (128, 1)).copy()

    # xT: full, s2 slot order, replicated
    xs = np.zeros((cfg.slots, cfg.f_in), np.float32)
    xs[perm2[:n]] = x
    xT_full = np.ascontiguousarray(xs.T).astype(npbf16)

    # xTo: own-core rows in original local tile order (for ald)
    xo = np.zeros((cfg.slots, cfg.f_in), np.float32)
    xo[perm[:n]] = x

    idx_lo_w = wrap16(idx_lo)
    idx_hi_w = wrap16(idx_hi)
    lo_cols = lo_slots // 16
    hi_cols = hi_slots // 16

    npc = cfg.nt
    in_maps = []
    for c in range(cfg.ncores):
        t0, t1 = c * npc, (c + 1) * npc
        m = {
            "xT": xT_full,
            "xTo": np.ascontiguousarray(
                xo[c * cfg.ntr:(c + 1) * cfg.ntr].T).astype(npbf16),
            "w1a": w1aug.astype(npbf16),
            "w2a": np.stack([w2aug[:128], w2aug[128:]]).astype(npbf16),
            "i128": i128,
            "iota": iotarow,
            "idxlo": np.ascontiguousarray(
                idx_lo_w[:, t0 * lo_cols:t1 * lo_cols]),
            "idxhi": np.ascontiguousarray(
                idx_hi_w[:, t0 * hi_cols:t1 * hi_cols]),
            "dstloc": np.ascontiguousarray(
                dst_loc[t0:t1].reshape(npc, cfg.cpt, 128)
                .transpose(2, 0, 1).reshape(128, npc * cfg.cpt)),
        }
        in_maps.append(m)
    return in_maps, perm


# ---------------------------------------------------------------------------
# Device program
# ---------------------------------------------------------------------------

def build_program(cfg):
    H, HID = cfg.heads, cfg.hid
    V1, V2 = cfg.v1, cfg.v2          # 256, 64
    T1W, T2W = cfg.t1w, cfg.t2w      # 384, 128
    NT, CPT, LOCH, HICH = cfg.nt, cfg.cpt, cfg.loch, cfg.hich
    NTR = cfg.ntr
    OUT = cfg.out_ch
    SPLIT = cfg.split
    K2 = H * HID                     # 256
    n_k2 = K2 // 128                 # 2
    NST = cfg.slots // 1024          # 49 supertiles in phase A
    W1C = V1 + 2 * H                 # 264
    AFT = mybir.ActivationFunctionType
    ALU = mybir.AluOpType

    nc = bacc.Bacc("TRN2", target_bir_lowering=False, debug=False,
                   num_devices=cfg.ncores)

    xT = nc.dram_tensor("xT", [cfg.f_in, cfg.slots], bf16, kind="ExternalInput")
    xTo_d = nc.dram_tensor("xTo", [cfg.f_in, NTR], bf16, kind="ExternalInput")
    w1a_d = nc.dram_tensor("w1a", [cfg.f_in, W1C], bf16, kind="ExternalInput")
    w2a_d = nc.dram_tensor("w2a", [n_k2, 128, V2 + 2], bf16, kind="ExternalInput")
    i128_d = nc.dram_tensor("i128", [128, 128], bf16, kind="ExternalInput")
    iota_d = nc.dram_tensor("iota", [128, 128], bf16, kind="ExternalInput")
    idxlo_d = nc.dram_tensor("idxlo", [128, NT * LOCH * 8], dt.int16, kind="ExternalInput")
    idxhi_d = nc.dram_tensor("idxhi", [128, NT * HICH * 8], dt.int16, kind="ExternalInput")
    dstloc_d = nc.dram_tensor("dstloc", [128, NT * CPT], f32, kind="ExternalInput")
    out_d = nc.dram_tensor("out_shard", [NTR, OUT], f32, kind="ExternalOutput")

    tbl1 = nc.dram_tensor("tbl1", [cfg.slots, T1W], bf16)
    tbl2_shard = nc.dram_tensor("tbl2_shard", [NTR, T2W], bf16)
    tbl2 = nc.dram_tensor("tbl2", [cfg.slots, T2W], bf16, addr_space="Shared")
    dbg = getattr(cfg, "debug", False)
    if dbg:
        tbl1_o = nc.dram_tensor("tbl1_dump", [cfg.slots, T1W], bf16,
                                kind="ExternalOutput")
        tbl2s_o = nc.dram_tensor("tbl2s_dump", [NTR, T2W], bf16,
                                 kind="ExternalOutput")
        tbl2_o = nc.dram_tensor("tbl2_dump", [cfg.slots, T2W], bf16,
                                kind="ExternalOutput")

    rg = [list(range(cfg.ncores))]
    AG2B = cfg.ag2_bounds

    with tile.TileContext(nc) as tc:
        with tc.tile_pool(name="res", bufs=1) as res:
            w1a = res.tile([cfg.f_in, W1C], bf16)
            w2a = res.tile([128, n_k2 * (V2 + 2)], bf16)
            i128 = res.tile([128, 128], bf16)
            iota = res.tile([128, 128], bf16)
            idxlo = res.tile([128, NT * LOCH * 8], dt.int16)
            idxhi = res.tile([128, NT * HICH * 8], dt.int16)
            dstloc = res.tile([128, NT * CPT], f32)
            xTo = res.tile([cfg.f_in, NTR], bf16)
            alds2 = res.tile([128, NT], bf16)
            clamp_c = res.tile([128, 1], f32)

            nc.sync.dma_start(w1a[:], w1a_d[:])
            for j in range(n_k2):
                nc.sync.dma_start(w2a[:, j * (V2 + 2):(j + 1) * (V2 + 2)],
                                  w2a_d[j, :, :])
            nc.sync.dma_start(i128[:], i128_d[:])
            nc.sync.dma_start(iota[:], iota_d[:])
            nc.sync.dma_start(idxlo[:], idxlo_d[:])
            nc.sync.dma_start(idxhi[:], idxhi_d[:])
            nc.sync.dma_start(dstloc[:], dstloc_d[:])
            nc.sync.dma_start(xTo[:], xTo_d[:])
            nc.gpsimd.memset(clamp_c[:], CLAMP)

            # ------------------------------------------------------------
            # Phase A (replicated): full table1 = [h1 | als], bf16
            # ------------------------------------------------------------
            with tc.tile_pool(name="pa_sb", bufs=2) as pa, \
                 tc.tile_pool(name="pa_ps", bufs=2, space="PSUM") as pap:
                for st in range(NST):
                    xg = pa.tile([128, 1024], bf16, tag="xg")
                    nc.sync.dma_start(xg[:], xT[:, st * 1024:(st + 1) * 1024])
                    stg = pa.tile([128, 8 * 260], bf16, tag="stg")
                    for sub in range(8):
                        ps = pap.tile([128, W1C], f32, tag="h1")
                        nc.tensor.matmul(ps[:], xg[:, sub * 128:(sub + 1) * 128],
                                         w1a[:], start=True, stop=True)
                        eng = nc.vector if sub % 8 < 3 else nc.scalar
                        if eng is nc.scalar:
                            eng.activation(stg[:, sub * 260:(sub + 1) * 260],
                                           ps[:, 0:260], AFT.Copy)
                        else:
                            eng.tensor_copy(stg[:, sub * 260:(sub + 1) * 260],
                                            ps[:, 0:260])
                    dst_ap = tbl1[st * 1024:(st + 1) * 1024, 0:260].rearrange(
                        "(s p) c -> p s c", p=128)
                    nc.sync.dma_start(
                        dst_ap, stg[:].rearrange("p (s c) -> p s c", c=260))

            # ------------------------------------------------------------
            # Edge phases
            # ------------------------------------------------------------
            def edge_phase(lay, tblw, tbl_full, nvals, nheads, evict_fn,
                           ag2_emit=None):
                with tc.tile_pool(name=f"eb{lay}", bufs=2) as eb, \
                     tc.tile_pool(name=f"oh{lay}", bufs=2) as ohp, \
                     tc.tile_pool(name=f"oht{lay}", bufs=2) as ohtp, \
                     tc.tile_pool(name=f"wx{lay}", bufs=2) as wxp, \
                     tc.tile_pool(name=f"ev{lay}", bufs=2) as ev, \
                     tc.tile_pool(name=f"ps{lay}", bufs=1, space="PSUM") as ps:
                    for t in range(NT):
                        if ag2_emit is not None:
                            ag2_emit(t)
                        gbuf = eb.tile([128, CPT * tblw], bf16, tag="gbuf")
                        g3 = gbuf[:].rearrange("p (c w) -> p c w", w=tblw)
                        lo_i = idxlo[:, t * LOCH * 8:(t + 1) * LOCH * 8]
                        hi_i = idxhi[:, t * HICH * 8:(t + 1) * HICH * 8]
                        nc.gpsimd.dma_gather(
                            g3[:, 0:LOCH, :], tbl_full[0:SPLIT, :],
                            lo_i, LOCH * 128, LOCH * 128, tblw,
                            single_packet=False)
                        nc.gpsimd.dma_gather(
                            g3[:, LOCH:CPT, :], tbl_full[SPLIT:cfg.slots, :],
                            hi_i, HICH * 128, HICH * 128, tblw,
                            single_packet=False)

                        # one-hots [e, d] (plain; L2 rescales into ohw later)
                        oh_all = ohp.tile([128, CPT * 128], bf16, tag="oh")
                        for cc in range(CPT):
                            nc.vector.tensor_scalar(
                                oh_all[:, cc * 128:(cc + 1) * 128], iota[:],
                                dstloc[:, t * CPT + cc:t * CPT + cc + 1], None,
                                ALU.is_equal)
                        # transposed one-hots [d, e] via PE; copies on ACT
                        ohT_all = ohtp.tile([128, CPT * 128], bf16, tag="ohT")
                        for cc in range(CPT):
                            ohT_ps = ps.tile([128, 128], bf16, tag="ohT", bufs=2)
                            nc.tensor.transpose(
                                ohT_ps[:], oh_all[:, cc * 128:(cc + 1) * 128],
                                i128[:])
                            nc.scalar.activation(
                                ohT_all[:, cc * 128:(cc + 1) * 128], ohT_ps[:],
                                AFT.Copy)

                        # al_d for this tile. ald and h2 (evict1) share one
                        # PSUM bank: their chains never overlap in time (PE
                        # executes in order), and each start=True clears only
                        # already-consumed has_written state.
                        if lay == 1:
                            misc = ps.tile([128, 80], f32, tag="misc", bufs=1)
                            ald_ps = misc[:, 0:H]
                            nc.tensor.matmul(
                                ald_ps, xTo[:, t * 128:(t + 1) * 128],
                                w1a[:, V1 + H:V1 + 2 * H], start=True, stop=True)
                            ald = ev.tile([128, H], bf16, tag="ald_sb")
                            nc.scalar.activation(ald[:], ald_ps, AFT.Copy)
                            ald_ap = ald[:]
                        else:
                            misc = None
                            ald_ap = alds2[:, t:t + 1]

                        # epre[e, cc, h] = als[src] + ald[dst]
                        epre_ps = ps.tile([128, CPT * nheads], f32, tag="epre",
                                          bufs=1)
                        ep3 = epre_ps[:].rearrange("p (c h) -> p c h", h=nheads)
                        aoff = nvals if lay == 1 else nvals + 1
                        als_view = g3[:, :, aoff:aoff + nheads]
                        nc.tensor.matmul(ep3, i128[:], als_view,
                                         start=True, stop=False,
                                         skip_group_check=True)
                        for cc in range(CPT):
                            nc.tensor.matmul(
                                ep3[:, cc, :],
                                ohT_all[:, cc * 128:(cc + 1) * 128], ald_ap,
                                start=False, stop=(cc == CPT - 1),
                                skip_group_check=True)

                        # w = exp(min(lrelu(epre), CLAMP)), exactly:
                        #   lr = Prelu(epre; 0.2); t1 = Relu(C - lr);
                        #   w  = Exp(C - t1)
                        nh = CPT * nheads
                        lr = wxp.tile([128, nh], f32, tag="lr")
                        nc.scalar.activation(lr[:], epre_ps[:], AFT.Prelu,
                                             alpha=NEG_SLOPE)
                        t1 = wxp.tile([128, nh], f32, tag="t1")
                        nc.scalar.activation(t1[:], lr[:], AFT.Relu,
                                             scale=-1.0, bias=clamp_c[:, 0:1])
                        w3f = wxp.tile([128, nh], f32, tag="w3f")
                        nc.scalar.activation(w3f[:], t1[:], AFT.Exp,
                                             scale=-1.0, bias=clamp_c[:, 0:1])
                        w3b = wxp.tile([128, nh], bf16, tag="w3b")
                        nc.scalar.activation(w3b[:], t1[:], AFT.Exp,
                                             scale=-1.0, bias=clamp_c[:, 0:1])
                        wf3 = w3f[:].rearrange("p (c h) -> p c h", h=nheads)
                        wb3 = w3b[:].rearrange("p (c h) -> p c h", h=nheads)

                        # NOTE: two interleaved accumulation chains must NOT
                        # share a PSUM bank -- start=True clears bank-wide.
                        seg_ps = ps.tile([128, nvals + nheads], f32, tag="seg",
                                         bufs=2)
                        if lay == 1:
                            den_ps = ps.tile([128, nheads], f32, tag="den",
                                             bufs=1)
                            # scale gathered values in place, then seg matmuls
                            # with the plain one-hot; den from extra matmul.
                            for cc in range(CPT):
                                gv = g3[:, cc, 0:nvals].rearrange(
                                    "p (h u) -> p h u", h=nheads)
                                wv = wf3[:, cc, :].unsqueeze(2).broadcast_to(
                                    [128, nheads, nvals // nheads])
                                nc.vector.tensor_tensor(gv, gv, wv, ALU.mult)
                                oh_cc = oh_all[:, cc * 128:(cc + 1) * 128]
                                nc.tensor.matmul(
                                    seg_ps[:, 0:nvals], oh_cc,
                                    g3[:, cc, 0:nvals],
                                    start=(cc == 0), stop=(cc == CPT - 1),
                                    skip_group_check=True)
                                nc.tensor.matmul(
                                    den_ps[:], oh_cc,
                                    wb3[:, cc, :],
                                    start=(cc == 0), stop=(cc == CPT - 1),
                                    skip_group_check=True)
                        else:
                            # fused weighted one-hot; table ones-col gives den
                            for cc in range(CPT):
                                ohw_cc = oh_all[:, cc * 128:(cc + 1) * 128]
                                nc.vector.tensor_scalar(
                                    ohw_cc, iota[:],
                                    dstloc[:, t * CPT + cc:t * CPT + cc + 1],
                                    wf3[:, cc, 0:1],
                                    ALU.is_equal, ALU.mult)
                                nc.tensor.matmul(
                                    seg_ps[:], ohw_cc,
                                    g3[:, cc, 0:nvals + 1],
                                    start=(cc == 0), stop=(cc == CPT - 1),
                                    skip_group_check=True)
                        evict_fn(t, seg_ps,
                                 den_ps if lay == 1 else None,
                                 (eb, ev, ps, misc))

            # ---- layer-1 eviction: ELU -> table2 rows ----
            def evict1(t, seg_ps, den_ps, pools):
                eb, ev, ps, misc = pools
                # den=0 on unused rows must become 1: a NaN row would poison
                # the L2 epre matmul through 0*NaN in the ald gather.
                denf = ev.tile([128, H], f32, tag="denf")
                nc.vector.tensor_scalar(denf[:], den_ps[:], 0.0,
                                        None, ALU.is_equal)
                nc.vector.tensor_tensor(denf[:], denf[:], den_ps[:],
                                        ALU.add)
                rec = ev.tile([128, H], f32, tag="rec")
                nc.vector.reciprocal(rec[:], denf[:])
                pe = ev.tile([128, K2], f32, tag="pelu")
                p3 = pe[:].rearrange("p (h u) -> p h u", h=H)
                sg3 = seg_ps[:, 0:V1].rearrange("p (h u) -> p h u", h=H)
                r3 = rec[:].unsqueeze(2).broadcast_to([128, H, HID])
                nc.vector.tensor_tensor(p3, sg3, r3, ALU.mult)
                # elu(v) = max(v,0) + exp(min(v,0)) - 1, built as
                #   mx = Relu(v); ex = Exp(-Relu(-v)); pre = (ex - 1) + mx
                mx = ev.tile([128, K2], f32, tag="mx")
                nc.scalar.activation(mx[:], pe[:], AFT.Relu)
                r1 = ev.tile([128, K2], f32, tag="r1")
                nc.scalar.activation(r1[:], pe[:], AFT.Relu, scale=-1.0)
                ex = ev.tile([128, K2], f32, tag="ex")
                nc.scalar.activation(ex[:], r1[:], AFT.Exp, scale=-1.0)
                pre = ev.tile([128, K2], bf16, tag="pre")
                nc.vector.scalar_tensor_tensor(pre[:], ex[:], -1.0, mx[:],
                                               ALU.add, ALU.add)
                # h2 rows: [h2 | als2 | ald2] = elu @ W2aug
                h2_ps = misc[:, 8:8 + V2 + 2]
                for j in range(n_k2):
                    peT_ps = ps.tile([128, 128], bf16, tag="peT", bufs=1)
                    nc.tensor.transpose(peT_ps[:], pre[:, j * 128:(j + 1) * 128],
                                        i128[:])
                    peT = ev.tile([128, 128], bf16, tag="peT_sb")
                    nc.scalar.activation(peT[:], peT_ps[:], AFT.Copy)
                    nc.tensor.matmul(h2_ps, peT[:],
                                     w2a[:, j * (V2 + 2):(j + 1) * (V2 + 2)],
                                     start=(j == 0), stop=(j == n_k2 - 1),
                                     skip_group_check=True)
                stg2 = eb.tile([128, T2W], bf16, tag="stg2")
                nc.scalar.activation(stg2[:, 0:V2], h2_ps[:, 0:V2], AFT.Copy)
                nc.gpsimd.memset(stg2[:, V2:V2 + 1], 1.0)
                nc.scalar.activation(stg2[:, V2 + 1:V2 + 2],
                                     h2_ps[:, V2:V2 + 1], AFT.Copy)
                nc.scalar.activation(alds2[:, t:t + 1],
                                     h2_ps[:, V2 + 1:V2 + 2], AFT.Copy)
                nc.sync.dma_start(tbl2_shard[t * 128:(t + 1) * 128, :], stg2[:])

            # chunked AllGather of tbl2, interleaved with L1 tiles. The s2
            # slot numbering makes each chunk's output a contiguous row
            # range of tbl2 -- no splice needed.
            def ag2_chunk(k):
                r0, r1 = AG2B[k] * 128, AG2B[k + 1] * 128
                nc.gpsimd.collective_compute(
                    "AllGather", mybir.AluOpType.bypass, replica_groups=rg,
                    ins=[tbl2_shard[r0:r1, :]],
                    outs=[tbl2[r0 * cfg.ncores:r1 * cfg.ncores, :]])

            def ag2_emit(t):
                for k in range(len(AG2B) - 2):
                    if t == AG2B[k + 1] + 2:
                        ag2_chunk(k)

            edge_phase(1, T1W, tbl1, V1, H, evict1, ag2_emit=ag2_emit)
            ag2_chunk(len(AG2B) - 2)

            # ---- layer-2 eviction: output rows ----
            def evict2(t, seg_ps, den_ps, pools):
                eb, ev, ps, misc = pools
                rec = ev.tile([128, 1], f32, tag="rec2")
                nc.vector.reciprocal(rec[:], seg_ps[:, OUT:OUT + 1])
                ot = ev.tile([128, OUT], f32, tag="ot")
                nc.vector.tensor_scalar(ot[:], seg_ps[:, 0:OUT], rec[:], None,
                                        ALU.mult)
                nc.sync.dma_start(out_d[t * 128:(t + 1) * 128, :], ot[:])

            edge_phase(2, T2W, tbl2, V2, 1, evict2)

            if dbg:
                nc.sync.dma_start(tbl1_o[:], tbl1[:])
                nc.sync.dma_start(tbl2s_o[:], tbl2_shard[:])
                nc.sync.dma_start(tbl2_o[:], tbl2[:])

    nc.compile()
    return nc


# ---------------------------------------------------------------------------
# Entry point
# ---------------------------------------------------------------------------

_CACHE = {}


def _get_program(cfg):
    key = tuple(sorted((k, v) for k, v in cfg.__dict__.items()
                       if not isinstance(v, tuple)))
    if key not in _CACHE:
        _CACHE[key] = build_program(cfg)
    return _CACHE[key]


def run(cfg, inputs, trace=False, **kw):
    in_maps, perm = prep_host(cfg, **inputs)
    nc = _get_program(cfg)
    res = run_bass_kernel_spmd(nc, in_maps, list(range(cfg.ncores)),
                               trace=trace, **kw)
    out_full = np.concatenate([res.results[c]["out_shard"]
                               for c in range(cfg.ncores)], axis=0)
    return out_full[perm[:cfg.n]].astype(np.float32), res


def kernel(x, edge_index, W1, a_src1, a_dst1, b1, W2, a_src2, a_dst2, b2):
    out, _ = run(FULL, dict(x=x, edge_index=edge_index, W1=W1,
                            a_src1=a_src1, a_dst1=a_dst1, b1=b1,
                            W2=W2, a_src2=a_src2, a_dst2=a_dst2, b2=b2))
    return out
